# revision 46
# baseline (speedup 1.0000x reference)
"""Trainium2 Bass kernel: 16-head attention (SEQ=4096, D_MODEL=1024, D_K=64).

Sharding: tensor-parallel over heads. 2 heads per core x 8 cores.
W_O is row-sharded; each core returns a partial [S, D] output projection,
summed on the host (the all-reduce of the output projection).

v2 dataflow (fp16-centric):
  qaug/kaug fp16 [65, S] per head: rows 0:64 = projections (f32r matmuls on
    fp32 Q^T/K^T slabs, fp16 store), row 64 = -max (q) / ones (k).
  v fp16 natural [s, dk] with ones column (att@V also yields row sums).
  Pass 1 (natural, fp16): scores for row maxes; DVE reduce (negate) ->
    fp16 partials -> +I transpose matmul -> qaug row 64.
  Pass 2 (transposed, fp16): K=65 matmul gives scores^T - max; both heads
    packed in one [128,1024] PSUM tile; ONE exp FD=1024 -> E fp16.
  [v|1] @ E accumulates att@V + row sums; normalize via reciprocal + ones
    replication matmul; W_O fp16 1024-wide; partial out DMA'd from PSUM.
"""

import os
import sys

import numpy as np

for _p in (
    "/root/.axon_site",
    "/root/.axon_site/_ro/trn_rl_repo",
    "/root/.axon_site/_ro/pypackages",
    "/opt/trn_rl_repo",
    "/opt/pypackages",
):
    if os.path.isdir(_p) and _p not in sys.path:
        sys.path.append(_p)

D = 1024
NHEADS = 16
DK = 64
NCORES = 8
S_FULL = 4096

_cache = {}
LAST_RESULT = None  # BassKernelResults of the most recent run (for test harness)
DEFER = True  # software-pipeline norm/W_O across tile boundaries
NAT_IN_PROJ = 2   # nat-t0 units per (g,d) step of q proj
NAT_PER_CHUNK = 2  # nat-t+1 units per main-loop chunk (rest at the boundary)
FAST_RECIP = True
FP16_QK = True  # load Q^T/K^T (and wq/wk) as fp16 instead of fp32/f32r


def _build(S):
    import concourse.bass as bass  # noqa: F401
    import concourse.tile as tile
    from concourse import bacc, mybir
    from concourse.masks import make_identity
    from contextlib import ExitStack

    f32 = mybir.dt.float32
    f32r = mybir.dt.float32r
    fp16 = mybir.dt.float16
    X = mybir.AxisListType.X
    Exp = mybir.ActivationFunctionType.Exp
    Max = mybir.AluOpType.max

    NT = S // 512   # 512-wide q tiles
    NCH = S // 128  # 128-wide s chunks
    ND = D // 128   # contraction chunks
    NB = S // 128   # 128-wide q blocks

    nc = bacc.Bacc(
        "TRN2",
        target_bir_lowering=False,
        debug=False,
        num_devices=NCORES,
    )
    qk_dt = fp16 if FP16_QK else f32r
    qt = nc.dram_tensor("qt", [D, S], qk_dt, kind="ExternalInput")
    kt = nc.dram_tensor("kt", [D, S], qk_dt, kind="ExternalInput")
    vt = nc.dram_tensor("vt", [D, S], fp16, kind="ExternalInput")
    wq = nc.dram_tensor("wq", [D, 128], qk_dt, kind="ExternalInput")
    wk = nc.dram_tensor("wk", [D, 128], qk_dt, kind="ExternalInput")
    wv = nc.dram_tensor("wv", [D, 128], fp16, kind="ExternalInput")
    wo = nc.dram_tensor("wo", [128, D], fp16, kind="ExternalInput")
    out = nc.dram_tensor("out", [S, D], fp16, kind="ExternalOutput")

    with tile.TileContext(nc) as tc, ExitStack() as ctx:
        consts = ctx.enter_context(tc.tile_pool(name="consts", bufs=1))
        big = ctx.enter_context(tc.tile_pool(name="big", bufs=1))
        ldpool = ctx.enter_context(tc.tile_pool(name="ld", bufs=8))
        epool = ctx.enter_context(tc.tile_pool(name="e", bufs=4))
        smalls = ctx.enter_context(tc.tile_pool(name="smalls", bufs=4))
        outp = ctx.enter_context(tc.tile_pool(name="outp", bufs=2))
        mxpool = ctx.enter_context(tc.tile_pool(name="mx", bufs=10))
        ps_nat = ctx.enter_context(tc.tile_pool(name="ps_nat", bufs=2, space="PSUM"))
        ps_fine = ctx.enter_context(tc.tile_pool(name="ps_fine", bufs=2, space="PSUM"))
        ps_av = ctx.enter_context(tc.tile_pool(name="ps_av", bufs=1, space="PSUM"))

        def pnat():
            return ps_nat.tile([128, 512], f32, tag="pn", name="pn")

        def pfine():
            return ps_fine.tile([128, 1024], f32, tag="pf", name="pf")

        # ---- constants
        ident_f = consts.tile([128, 128], f32)
        make_identity(nc, ident_f)
        identh = consts.tile([128, 128], fp16)
        nc.vector.tensor_copy(identh[:], ident_f[:])
        ones64 = consts.tile([1, 64], fp16)
        nc.vector.memset(ones64[:], 1.0)

        # ---- weights
        wq_sb = consts.tile([128, ND, 128], qk_dt)
        nc.sync.dma_start(wq_sb[:], wq.rearrange("(o p) f -> p o f", p=128))
        wk_sb = consts.tile([128, ND, 128], qk_dt)
        nc.sync.dma_start(wk_sb[:], wk.rearrange("(o p) f -> p o f", p=128))
        wv_sb = consts.tile([128, ND, 128], fp16)
        nc.sync.dma_start(wv_sb[:], wv.rearrange("(o p) f -> p o f", p=128))
        wo_sb = consts.tile([128, D], fp16)
        nc.sync.dma_start(wo_sb[:], wo[:])

        # ---- big SBUF tensors
        # qaug/kaug padded to 128 partitions (rows 65..127 zero) so the fine
        # matmuls have a full-128 contraction -> FWL-eligible LDWEIGHTS.
        qaug = [big.tile([128, S], fp16, tag=f"qaug{h}", name=f"qaug{h}") for h in range(2)]
        kaug = [big.tile([128, S], fp16, tag=f"kaug{h}", name=f"kaug{h}") for h in range(2)]
        for h in range(2):
            nc.vector.memset(qaug[h][64:128, :], 0.0)
            nc.vector.memset(kaug[h][64:128, :], 0.0)
        v_sb = big.tile([128, NCH, 2, 65], fp16, tag="v", name="v_sb")
        concat = big.tile([128, S], fp16, tag="concat", name="concat")
        vt_sb = big.tile([128, ND, S], fp16, tag="vt_sb", name="vt_sb")
        for h in range(2):
            nc.vector.memset(kaug[h][64:65, :], 1.0)
        nc.vector.memset(v_sb[:, :, :, 64:65], 1.0)

        # ---- natural pass units (row maxes for softmax shift)
        mxs_all = {}

        def nat_unit(b, h, sh):
            qsl = slice(b * 128, (b + 1) * 128)
            ssl = slice(sh * 512, (sh + 1) * 512)
            psn = pnat()
            nc.tensor.matmul(psn[:], qaug[h][0:64, qsl],
                             kaug[h][0:64, ssl], start=True, stop=True)
            if (b, h) not in mxs_all:
                mxs_all[(b, h)] = mxpool.tile([128, S // 512], fp16,
                                              tag="mx", name="mx")
            nc.vector.tensor_reduce(mxs_all[(b, h)][:, sh:sh + 1], psn[:],
                                    axis=X, op=Max)

        def nat_finish(b, h):
            qsl = slice(b * 128, (b + 1) * 128)
            m2 = smalls.tile([128, 1], fp16, tag="m2", name="m2")
            nc.vector.tensor_reduce(m2[:], mxs_all.pop((b, h))[:],
                                    axis=X, op=Max, negate=True)
            psmt = pnat()
            nc.tensor.matmul(psmt[0:1, 0:128], m2[:], identh[:],
                             start=True, stop=True)
            nc.scalar.copy(qaug[h][64:65, qsl], psmt[0:1, 0:128])

        # Tile-0 natural pass ordered sh-outer so each 16-unit batch only
        # needs the kaug column group that has already streamed in.
        NAT0 = [(b, h, sh) for sh in range(S // 512) for b in range(4)
                for h in range(2)]
        n0 = 0

        def emit_nat0(k):
            nonlocal n0
            for _ in range(k):
                if n0 < len(NAT0):
                    b, h, sh = NAT0[n0]
                    nat_unit(b, h, sh)
                    n0 += 1
                    if n0 == len(NAT0):
                        for bb in range(4):
                            for hh in range(2):
                                nat_finish(bb, hh)

        # ---- projections.  Stream fp32 slabs in [128, 1024] slices (4KB
        # rows -> full DMA BW); accumulate into [128,1024] PSUM (f32r
        # matmuls are 512-wide max); store fp16 into qaug/kaug rows 0:64.
        # DMA queue order: kt g0, qt g0, kt g1-3, vt, qt g1 -- only
        # kt + qt g0 + vt (29MB) gate the main loop.  qt groups 1..3 are
        # projected in bursts at the first three tile boundaries, their
        # loads streaming underneath the main loop.
        def proj_step(src, wsb, dstA, dstB, g, d, ps, lt=None):
            gsl = slice(g * 1024, (g + 1) * 1024)
            if lt is None:
                lt = ldpool.tile([128, 1024], qk_dt, tag="ld", name="ld")
                nc.sync.dma_start(lt[:], src[d * 128:(d + 1) * 128, gsl])
            nc.tensor.matmul(ps[:, 0:512], wsb[:, d, :], lt[:, 0:512],
                             start=(d == 0), stop=(d == ND - 1),
                             skip_group_check=True)
            nc.tensor.matmul(ps[:, 512:1024], wsb[:, d, :], lt[:, 512:1024],
                             start=(d == 0), stop=(d == ND - 1),
                             skip_group_check=True)
            if d == ND - 1:
                nc.scalar.copy(dstA[0:64, gsl], ps[0:64, :])
                nc.scalar.copy(dstB[0:64, gsl], ps[64:128, :])

        psk = pfine()
        for d in range(ND):
            proj_step(kt, wk_sb, kaug[0], kaug[1], 0, d, psk)
        psq = pfine()
        for d in range(ND):
            proj_step(qt, wq_sb, qaug[0], qaug[1], 0, d, psq)
        for g in range(1, S // 1024):
            psk = pfine()
            for d in range(ND):
                proj_step(kt, wk_sb, kaug[0], kaug[1], g, d, psk)
                emit_nat0(2)  # sh batches track the landed kaug groups

        qpre = {}

        def q_prefetch(g):
            tiles = []
            gsl = slice(g * 1024, (g + 1) * 1024)
            for d in range(ND):
                lt = ldpool.tile([128, 1024], qk_dt, tag="ld", name="ld")
                nc.sync.dma_start(lt[:], qt[d * 128:(d + 1) * 128, gsl])
                tiles.append(lt)
            qpre[g] = tiles

        def q_burst(g):
            ps = pfine()
            tiles = qpre.pop(g)
            for d in range(ND):
                proj_step(qt, wq_sb, qaug[0], qaug[1], g, d, ps, lt=tiles[d])

        # V load (8 slab DMAs), qt group-1 prefetch loads, then the v
        # projection interleaved with the rest of tile-0's natural pass.
        for d in range(ND):
            nc.sync.dma_start(vt_sb[:, d, :], vt[d * 128:(d + 1) * 128, :])
        q_prefetch(1)
        for sb in range(NB):
            psv = pnat()
            for d in range(ND):
                nc.tensor.matmul(psv[:, 0:128],
                                 vt_sb[:, d, sb * 128:(sb + 1) * 128],
                                 wv_sb[:, d, :],
                                 start=(d == 0), stop=(d == ND - 1),
                                 skip_group_check=True)
            nc.vector.tensor_copy(
                v_sb[:, sb, :, 0:64],
                psv[:, 0:128].rearrange("p (h f) -> p h f", h=2))
            emit_nat0(1)
        emit_nat0(len(NAT0))

        # ---- main loop over q tiles, software-pipelined boundaries.
        # At the end of tile t the av accumulators are copied to SBUF in one
        # fast DVE op each (freeing the PSUM banks for tile t+1); the
        # normalization chain and W_O matmuls for tile t are emitted during
        # tile t+1's early chunks so the PE never waits on the DVE.
        avpool = ctx.enter_context(tc.tile_pool(name="avb", bufs=2))

        def norm_emit(t, avbs):
            tsl = slice(t * 512, (t + 1) * 512)
            for h in range(2):
                avb = avbs[h]
                # reciprocal_approx_fast needs a partition-0 operand
                sums = smalls.tile([1, 512], f32, tag="sums", name="sums")
                nc.vector.tensor_copy(sums[:], avb[64:65, :])
                rec = smalls.tile([1, 512], f32, tag="rec", name="rec")
                if FAST_RECIP:
                    nc.vector.reciprocal_approx_fast(rec[:], sums[:])
                else:
                    nc.vector.reciprocal(rec[:], sums[:])
                rec_r = smalls.tile([1, 512], fp16, tag="rec_r", name="rec_r")
                nc.vector.tensor_copy(rec_r[:], rec[:])
                psr = pnat()
                nc.tensor.matmul(psr[0:64, :], ones64[:], rec_r[:],
                                 start=True, stop=True)
                reps = smalls.tile([64, 512], fp16, tag="reps", name="reps")
                nc.vector.tensor_copy(reps[:], psr[0:64, :])
                nc.vector.tensor_mul(concat[h * 64:(h + 1) * 64, tsl],
                                     avb[0:64, :], reps[:])

        def wo_emit(qb):
            pso = pfine()
            for n in range(2):
                nc.tensor.matmul(pso[:, n * 512:(n + 1) * 512],
                                 concat[:, qb * 128:(qb + 1) * 128],
                                 wo_sb[:, n * 512:(n + 1) * 512],
                                 start=True, stop=True,
                                 skip_group_check=True)
            ot = outp.tile([128, 1024], fp16, tag="ot", name="ot")
            if qb % 2 == 0:
                nc.scalar.copy(ot[:], pso[:])
            else:
                nc.vector.tensor_copy(ot[:], pso[:])
            nc.sync.dma_start(out[qb * 128:(qb + 1) * 128, :], ot[:])

        prev = None  # (t, avbs) of the previous tile
        for t in range(NT):
            tsl = slice(t * 512, (t + 1) * 512)
            psAs = [ps_av.tile([65, 512], f32, tag=f"pav{h}", name=f"pav{h}")
                    for h in range(2)]
            es = {}
            # nat units for tile t+1, 3 per chunk (finish well before c=31)
            nxt = [] if t + 1 >= NT else [
                (b, h, sh) for b in range(4 * (t + 1), 4 * (t + 2))
                for h in range(2) for sh in range(S // 512)]
            ni = 0
            for c in range(NCH):
                psF = pfine()
                nc.tensor.matmul(psF[:, 0:512],
                                 kaug[0][:, c * 128:(c + 1) * 128],
                                 qaug[0][:, tsl],
                                 start=True, stop=True, skip_group_check=True)
                nc.tensor.matmul(psF[:, 512:1024],
                                 kaug[1][:, c * 128:(c + 1) * 128],
                                 qaug[1][:, tsl],
                                 start=True, stop=True, skip_group_check=True)
                if DEFER and c == 0 and prev is not None:
                    norm_emit(*prev)
                if c > 0:
                    eprev = es.pop(c - 1)
                    for h in range(2):
                        nc.tensor.matmul(psAs[h][:],
                                         v_sb[:, c - 1, h, :],
                                         eprev[:, h * 512:(h + 1) * 512],
                                         start=(c - 1 == 0), stop=False,
                                         skip_group_check=True)
                e = epool.tile([128, 1024], fp16, tag="e", name="e")
                nc.scalar.activation(e[:], psF[:], Exp)
                es[c] = e

                # stream next tile's natural pass (the last few units are
                # held for the tile boundary to keep the PE dense there,
                # avoiding a HAM re-throttle)
                for _ in range(NAT_PER_CHUNK if c < NCH - 3 else 0):
                    if ni < len(nxt):
                        b, h, sh = nxt[ni]
                        nat_unit(b, h, sh)
                        ni += 1
                        if sh == S // 512 - 1:
                            nat_finish(b, h)
            elast = es.pop(NCH - 1)
            for h in range(2):
                nc.tensor.matmul(psAs[h][:], v_sb[:, NCH - 1, h, :],
                                 elast[:, h * 512:(h + 1) * 512],
                                 start=(NCH - 1 == 0), stop=True,
                                 skip_group_check=True)
            # free the av PSUM banks with one copy per head
            avbs = []
            for h in range(2):
                avb = avpool.tile([65, 512], f32, tag=f"avb{h}", name=f"avb{h}")
                nc.vector.tensor_copy(avb[:], psAs[h][:])
                avbs.append(avb)
            # boundary PE work: W_O for tile t-1 (DVE-independent, keeps the
            # PE dense through the boundary so HAM stays warm) interleaved
            # with the held-back natural-pass units (DVE-paced)
            wo_t = prev[0] if (DEFER and prev is not None) else None
            prev = (t, avbs)
            bi = 0
            while ni < len(nxt) or (wo_t is not None and bi < 4):
                if wo_t is not None and bi < 4:
                    wo_emit(wo_t * 4 + bi)
                    bi += 1
                if ni < len(nxt):
                    b, h, sh = nxt[ni]
                    nat_unit(b, h, sh)
                    ni += 1
                    if sh == S // 512 - 1:
                        nat_finish(b, h)
            # q-projection burst for a later column group: PE work that
            # covers the tile-boundary DVE chain; loads stream under tile t+1
            if t + 1 in qpre:
                q_burst(t + 1)
                if t + 2 < S // 1024:
                    q_prefetch(t + 2)
            if not DEFER:
                norm_emit(*prev)
                for b in range(4):
                    wo_emit(t * 4 + b)

        if DEFER:
            norm_emit(*prev)
            for b in range(4):
                wo_emit(prev[0] * 4 + b)

    nc.compile()
    return nc


def _prep_inputs(Q, K, V, W_Q, W_K, W_V, W_O):
    Q = np.asarray(Q, dtype=np.float32)
    K = np.asarray(K, dtype=np.float32)
    V = np.asarray(V, dtype=np.float32)
    W_Q = np.asarray(W_Q, dtype=np.float32)
    W_K = np.asarray(W_K, dtype=np.float32)
    W_V = np.asarray(W_V, dtype=np.float32)
    W_O = np.asarray(W_O, dtype=np.float32)

    qk_np = np.float16 if FP16_QK else np.float32
    QT = np.ascontiguousarray(Q.T.astype(qk_np))
    KT = np.ascontiguousarray(K.T.astype(qk_np))
    VT = np.ascontiguousarray(V.T.astype(np.float16))
    scale = np.float32(0.125)  # 1/sqrt(64), exact power of two

    in_maps = []
    for c in range(NCORES):
        hA, hB = 2 * c, 2 * c + 1
        in_maps.append({
            "qt": QT,
            "kt": KT,
            "vt": VT,
            "wq": np.ascontiguousarray(
                np.concatenate([W_Q[hA], W_Q[hB]], axis=1).astype(qk_np)),
            "wk": np.ascontiguousarray(
                np.concatenate([W_K[hA] * scale, W_K[hB] * scale],
                               axis=1).astype(qk_np)),
            "wv": np.ascontiguousarray(
                np.concatenate([W_V[hA], W_V[hB]], axis=1).astype(np.float16)),
            "wo": np.ascontiguousarray(
                W_O[c * 128:(c + 1) * 128, :].astype(np.float16)),
        })
    return in_maps


def kernel(Q, K, V, W_Q, W_K, W_V, W_O):
    global LAST_RESULT
    from concourse.bass_utils import run_bass_kernel_spmd

    S = np.asarray(Q).shape[0]
    nc = _cache.get(S)
    if nc is None:
        nc = _build(S)
        _cache[S] = nc

    in_maps = _prep_inputs(Q, K, V, W_Q, W_K, W_V, W_O)
    res = run_bass_kernel_spmd(nc, in_maps, list(range(NCORES)))
    LAST_RESULT = res
    parts = np.stack([res.results[i]["out"] for i in range(NCORES)])
    return parts.sum(axis=0, dtype=np.float32)


# revision 47
# speedup vs baseline: 1.0023x; 1.0023x over previous
"""Trainium2 Bass kernel: 16-head attention (SEQ=4096, D_MODEL=1024, D_K=64).

Sharding: tensor-parallel over heads. 2 heads per core x 8 cores.
W_O is row-sharded; each core returns a partial [S, D] output projection,
summed on the host (the all-reduce of the output projection).

v2 dataflow (fp16-centric):
  qaug/kaug fp16 [65, S] per head: rows 0:64 = projections (f32r matmuls on
    fp32 Q^T/K^T slabs, fp16 store), row 64 = -max (q) / ones (k).
  v fp16 natural [s, dk] with ones column (att@V also yields row sums).
  Pass 1 (natural, fp16): scores for row maxes; DVE reduce (negate) ->
    fp16 partials -> +I transpose matmul -> qaug row 64.
  Pass 2 (transposed, fp16): K=65 matmul gives scores^T - max; both heads
    packed in one [128,1024] PSUM tile; ONE exp FD=1024 -> E fp16.
  [v|1] @ E accumulates att@V + row sums; normalize via reciprocal + ones
    replication matmul; W_O fp16 1024-wide; partial out DMA'd from PSUM.
"""

import os
import sys

import numpy as np

for _p in (
    "/root/.axon_site",
    "/root/.axon_site/_ro/trn_rl_repo",
    "/root/.axon_site/_ro/pypackages",
    "/opt/trn_rl_repo",
    "/opt/pypackages",
):
    if os.path.isdir(_p) and _p not in sys.path:
        sys.path.append(_p)

D = 1024
NHEADS = 16
DK = 64
NCORES = 8
S_FULL = 4096

_cache = {}
LAST_RESULT = None  # BassKernelResults of the most recent run (for test harness)
DEFER = True  # software-pipeline norm/W_O across tile boundaries
NAT_IN_PROJ = 2   # nat-t0 units per (g,d) step of q proj
NAT_PER_CHUNK = 3  # nat-t+1 units per main-loop chunk (rest at the boundary)
FAST_RECIP = True
FP16_QK = True  # load Q^T/K^T (and wq/wk) as fp16 instead of fp32/f32r


def _build(S):
    import concourse.bass as bass  # noqa: F401
    import concourse.tile as tile
    from concourse import bacc, mybir
    from concourse.masks import make_identity
    from contextlib import ExitStack

    f32 = mybir.dt.float32
    f32r = mybir.dt.float32r
    fp16 = mybir.dt.float16
    X = mybir.AxisListType.X
    Exp = mybir.ActivationFunctionType.Exp
    Max = mybir.AluOpType.max

    NT = S // 512   # 512-wide q tiles
    NCH = S // 128  # 128-wide s chunks
    ND = D // 128   # contraction chunks
    NB = S // 128   # 128-wide q blocks

    nc = bacc.Bacc(
        "TRN2",
        target_bir_lowering=False,
        debug=False,
        num_devices=NCORES,
    )
    qk_dt = fp16 if FP16_QK else f32r
    qt = nc.dram_tensor("qt", [D, S], qk_dt, kind="ExternalInput")
    kt = nc.dram_tensor("kt", [D, S], qk_dt, kind="ExternalInput")
    vt = nc.dram_tensor("vt", [D, S], fp16, kind="ExternalInput")
    wq = nc.dram_tensor("wq", [D, 128], qk_dt, kind="ExternalInput")
    wk = nc.dram_tensor("wk", [D, 128], qk_dt, kind="ExternalInput")
    wv = nc.dram_tensor("wv", [D, 128], fp16, kind="ExternalInput")
    wo = nc.dram_tensor("wo", [128, D], fp16, kind="ExternalInput")
    out = nc.dram_tensor("out", [S, D], fp16, kind="ExternalOutput")

    with tile.TileContext(nc) as tc, ExitStack() as ctx:
        consts = ctx.enter_context(tc.tile_pool(name="consts", bufs=1))
        big = ctx.enter_context(tc.tile_pool(name="big", bufs=1))
        ldpool = ctx.enter_context(tc.tile_pool(name="ld", bufs=8))
        epool = ctx.enter_context(tc.tile_pool(name="e", bufs=4))
        smalls = ctx.enter_context(tc.tile_pool(name="smalls", bufs=4))
        outp = ctx.enter_context(tc.tile_pool(name="outp", bufs=2))
        mxpool = ctx.enter_context(tc.tile_pool(name="mx", bufs=10))
        ps_nat = ctx.enter_context(tc.tile_pool(name="ps_nat", bufs=2, space="PSUM"))
        ps_fine = ctx.enter_context(tc.tile_pool(name="ps_fine", bufs=2, space="PSUM"))
        ps_av = ctx.enter_context(tc.tile_pool(name="ps_av", bufs=1, space="PSUM"))

        def pnat():
            return ps_nat.tile([128, 512], f32, tag="pn", name="pn")

        def pfine():
            return ps_fine.tile([128, 1024], f32, tag="pf", name="pf")

        # ---- constants
        ident_f = consts.tile([128, 128], f32)
        make_identity(nc, ident_f)
        identh = consts.tile([128, 128], fp16)
        nc.vector.tensor_copy(identh[:], ident_f[:])
        ones64 = consts.tile([1, 64], fp16)
        nc.vector.memset(ones64[:], 1.0)

        # ---- weights
        wq_sb = consts.tile([128, ND, 128], qk_dt)
        nc.sync.dma_start(wq_sb[:], wq.rearrange("(o p) f -> p o f", p=128))
        wk_sb = consts.tile([128, ND, 128], qk_dt)
        nc.sync.dma_start(wk_sb[:], wk.rearrange("(o p) f -> p o f", p=128))
        wv_sb = consts.tile([128, ND, 128], fp16)
        nc.sync.dma_start(wv_sb[:], wv.rearrange("(o p) f -> p o f", p=128))
        wo_sb = consts.tile([128, D], fp16)
        nc.sync.dma_start(wo_sb[:], wo[:])

        # ---- big SBUF tensors
        # qaug/kaug padded to 128 partitions (rows 65..127 zero) so the fine
        # matmuls have a full-128 contraction -> FWL-eligible LDWEIGHTS.
        qaug = [big.tile([128, S], fp16, tag=f"qaug{h}", name=f"qaug{h}") for h in range(2)]
        kaug = [big.tile([128, S], fp16, tag=f"kaug{h}", name=f"kaug{h}") for h in range(2)]
        for h in range(2):
            nc.vector.memset(qaug[h][64:128, :], 0.0)
            nc.vector.memset(kaug[h][64:128, :], 0.0)
        v_sb = big.tile([128, NCH, 2, 65], fp16, tag="v", name="v_sb")
        concat = big.tile([128, S], fp16, tag="concat", name="concat")
        vt_sb = big.tile([128, ND, S], fp16, tag="vt_sb", name="vt_sb")
        for h in range(2):
            nc.vector.memset(kaug[h][64:65, :], 1.0)
        nc.vector.memset(v_sb[:, :, :, 64:65], 1.0)

        # ---- natural pass units (row maxes for softmax shift)
        mxs_all = {}

        def nat_unit(b, h, sh):
            qsl = slice(b * 128, (b + 1) * 128)
            ssl = slice(sh * 512, (sh + 1) * 512)
            psn = pnat()
            nc.tensor.matmul(psn[:], qaug[h][0:64, qsl],
                             kaug[h][0:64, ssl], start=True, stop=True)
            if (b, h) not in mxs_all:
                mxs_all[(b, h)] = mxpool.tile([128, S // 512], fp16,
                                              tag="mx", name="mx")
            nc.vector.tensor_reduce(mxs_all[(b, h)][:, sh:sh + 1], psn[:],
                                    axis=X, op=Max)

        def nat_finish(b, h):
            qsl = slice(b * 128, (b + 1) * 128)
            m2 = smalls.tile([128, 1], fp16, tag="m2", name="m2")
            nc.vector.tensor_reduce(m2[:], mxs_all.pop((b, h))[:],
                                    axis=X, op=Max, negate=True)
            psmt = pnat()
            nc.tensor.matmul(psmt[0:1, 0:128], m2[:], identh[:],
                             start=True, stop=True)
            nc.scalar.copy(qaug[h][64:65, qsl], psmt[0:1, 0:128])

        # Tile-0 natural pass ordered sh-outer so each 16-unit batch only
        # needs the kaug column group that has already streamed in.
        NAT0 = [(b, h, sh) for sh in range(S // 512) for b in range(4)
                for h in range(2)]
        n0 = 0

        def emit_nat0(k):
            nonlocal n0
            for _ in range(k):
                if n0 < len(NAT0):
                    b, h, sh = NAT0[n0]
                    nat_unit(b, h, sh)
                    n0 += 1
                    if n0 == len(NAT0):
                        for bb in range(4):
                            for hh in range(2):
                                nat_finish(bb, hh)

        # ---- projections.  Stream fp32 slabs in [128, 1024] slices (4KB
        # rows -> full DMA BW); accumulate into [128,1024] PSUM (f32r
        # matmuls are 512-wide max); store fp16 into qaug/kaug rows 0:64.
        # DMA queue order: kt g0, qt g0, kt g1-3, vt, qt g1 -- only
        # kt + qt g0 + vt (29MB) gate the main loop.  qt groups 1..3 are
        # projected in bursts at the first three tile boundaries, their
        # loads streaming underneath the main loop.
        def proj_step(src, wsb, dstA, dstB, g, d, ps, lt=None):
            gsl = slice(g * 1024, (g + 1) * 1024)
            if lt is None:
                lt = ldpool.tile([128, 1024], qk_dt, tag="ld", name="ld")
                nc.sync.dma_start(lt[:], src[d * 128:(d + 1) * 128, gsl])
            nc.tensor.matmul(ps[:, 0:512], wsb[:, d, :], lt[:, 0:512],
                             start=(d == 0), stop=(d == ND - 1),
                             skip_group_check=True)
            nc.tensor.matmul(ps[:, 512:1024], wsb[:, d, :], lt[:, 512:1024],
                             start=(d == 0), stop=(d == ND - 1),
                             skip_group_check=True)
            if d == ND - 1:
                nc.scalar.copy(dstA[0:64, gsl], ps[0:64, :])
                nc.scalar.copy(dstB[0:64, gsl], ps[64:128, :])

        psk = pfine()
        for d in range(ND):
            proj_step(kt, wk_sb, kaug[0], kaug[1], 0, d, psk)
        psq = pfine()
        for d in range(ND):
            proj_step(qt, wq_sb, qaug[0], qaug[1], 0, d, psq)
        for g in range(1, S // 1024):
            psk = pfine()
            for d in range(ND):
                proj_step(kt, wk_sb, kaug[0], kaug[1], g, d, psk)
                emit_nat0(2)  # sh batches track the landed kaug groups

        qpre = {}

        def q_prefetch(g):
            tiles = []
            gsl = slice(g * 1024, (g + 1) * 1024)
            for d in range(ND):
                lt = ldpool.tile([128, 1024], qk_dt, tag="ld", name="ld")
                nc.sync.dma_start(lt[:], qt[d * 128:(d + 1) * 128, gsl])
                tiles.append(lt)
            qpre[g] = tiles

        def q_burst(g):
            ps = pfine()
            tiles = qpre.pop(g)
            for d in range(ND):
                proj_step(qt, wq_sb, qaug[0], qaug[1], g, d, ps, lt=tiles[d])

        # V load (8 slab DMAs), qt group-1 prefetch loads, then the v
        # projection interleaved with the rest of tile-0's natural pass.
        for d in range(ND):
            nc.sync.dma_start(vt_sb[:, d, :], vt[d * 128:(d + 1) * 128, :])
        q_prefetch(1)
        for sb in range(NB):
            psv = pnat()
            for d in range(ND):
                nc.tensor.matmul(psv[:, 0:128],
                                 vt_sb[:, d, sb * 128:(sb + 1) * 128],
                                 wv_sb[:, d, :],
                                 start=(d == 0), stop=(d == ND - 1),
                                 skip_group_check=True)
            nc.vector.tensor_copy(
                v_sb[:, sb, :, 0:64],
                psv[:, 0:128].rearrange("p (h f) -> p h f", h=2))
            emit_nat0(1)
        emit_nat0(len(NAT0))

        # ---- main loop over q tiles, software-pipelined boundaries.
        # At the end of tile t the av accumulators are copied to SBUF in one
        # fast DVE op each (freeing the PSUM banks for tile t+1); the
        # normalization chain and W_O matmuls for tile t are emitted during
        # tile t+1's early chunks so the PE never waits on the DVE.
        avpool = ctx.enter_context(tc.tile_pool(name="avb", bufs=2))

        def norm_emit(t, avbs):
            tsl = slice(t * 512, (t + 1) * 512)
            for h in range(2):
                avb = avbs[h]
                # reciprocal_approx_fast needs a partition-0 operand
                sums = smalls.tile([1, 512], f32, tag="sums", name="sums")
                nc.vector.tensor_copy(sums[:], avb[64:65, :])
                rec = smalls.tile([1, 512], f32, tag="rec", name="rec")
                if FAST_RECIP:
                    nc.vector.reciprocal_approx_fast(rec[:], sums[:])
                else:
                    nc.vector.reciprocal(rec[:], sums[:])
                rec_r = smalls.tile([1, 512], fp16, tag="rec_r", name="rec_r")
                nc.vector.tensor_copy(rec_r[:], rec[:])
                psr = pnat()
                nc.tensor.matmul(psr[0:64, :], ones64[:], rec_r[:],
                                 start=True, stop=True)
                reps = smalls.tile([64, 512], fp16, tag="reps", name="reps")
                nc.vector.tensor_copy(reps[:], psr[0:64, :])
                nc.vector.tensor_mul(concat[h * 64:(h + 1) * 64, tsl],
                                     avb[0:64, :], reps[:])

        def wo_emit(qb):
            pso = pfine()
            for n in range(2):
                nc.tensor.matmul(pso[:, n * 512:(n + 1) * 512],
                                 concat[:, qb * 128:(qb + 1) * 128],
                                 wo_sb[:, n * 512:(n + 1) * 512],
                                 start=True, stop=True,
                                 skip_group_check=True)
            ot = outp.tile([128, 1024], fp16, tag="ot", name="ot")
            if qb % 2 == 0:
                nc.scalar.copy(ot[:], pso[:])
            else:
                nc.vector.tensor_copy(ot[:], pso[:])
            nc.sync.dma_start(out[qb * 128:(qb + 1) * 128, :], ot[:])

        prev = None  # (t, avbs) of the previous tile
        for t in range(NT):
            tsl = slice(t * 512, (t + 1) * 512)
            psAs = [ps_av.tile([65, 512], f32, tag=f"pav{h}", name=f"pav{h}")
                    for h in range(2)]
            es = {}
            # nat units for tile t+1, 3 per chunk (finish well before c=31)
            nxt = [] if t + 1 >= NT else [
                (b, h, sh) for b in range(4 * (t + 1), 4 * (t + 2))
                for h in range(2) for sh in range(S // 512)]
            ni = 0
            for c in range(NCH):
                psF = pfine()
                nc.tensor.matmul(psF[:, 0:512],
                                 kaug[0][:, c * 128:(c + 1) * 128],
                                 qaug[0][:, tsl],
                                 start=True, stop=True, skip_group_check=True)
                nc.tensor.matmul(psF[:, 512:1024],
                                 kaug[1][:, c * 128:(c + 1) * 128],
                                 qaug[1][:, tsl],
                                 start=True, stop=True, skip_group_check=True)
                if DEFER and c == 0 and prev is not None:
                    norm_emit(*prev)
                if c > 0:
                    eprev = es.pop(c - 1)
                    for h in range(2):
                        nc.tensor.matmul(psAs[h][:],
                                         v_sb[:, c - 1, h, :],
                                         eprev[:, h * 512:(h + 1) * 512],
                                         start=(c - 1 == 0), stop=False,
                                         skip_group_check=True)
                e = epool.tile([128, 1024], fp16, tag="e", name="e")
                nc.scalar.activation(e[:], psF[:], Exp)
                es[c] = e

                # stream next tile's natural pass (the last few units are
                # held for the tile boundary to keep the PE dense there,
                # avoiding a HAM re-throttle)
                for _ in range(NAT_PER_CHUNK if c < NCH - 3 else 0):
                    if ni < len(nxt):
                        b, h, sh = nxt[ni]
                        nat_unit(b, h, sh)
                        ni += 1
                        if sh == S // 512 - 1:
                            nat_finish(b, h)
            elast = es.pop(NCH - 1)
            for h in range(2):
                nc.tensor.matmul(psAs[h][:], v_sb[:, NCH - 1, h, :],
                                 elast[:, h * 512:(h + 1) * 512],
                                 start=(NCH - 1 == 0), stop=True,
                                 skip_group_check=True)
            # free the av PSUM banks with one copy per head
            avbs = []
            for h in range(2):
                avb = avpool.tile([65, 512], f32, tag=f"avb{h}", name=f"avb{h}")
                nc.vector.tensor_copy(avb[:], psAs[h][:])
                avbs.append(avb)
            # boundary PE work: W_O for tile t-1 (DVE-independent, keeps the
            # PE dense through the boundary so HAM stays warm) interleaved
            # with the held-back natural-pass units (DVE-paced)
            wo_t = prev[0] if (DEFER and prev is not None) else None
            prev = (t, avbs)
            bi = 0
            while ni < len(nxt) or (wo_t is not None and bi < 4):
                if wo_t is not None and bi < 4:
                    wo_emit(wo_t * 4 + bi)
                    bi += 1
                if ni < len(nxt):
                    b, h, sh = nxt[ni]
                    nat_unit(b, h, sh)
                    ni += 1
                    if sh == S // 512 - 1:
                        nat_finish(b, h)
            # q-projection burst for a later column group: PE work that
            # covers the tile-boundary DVE chain; loads stream under tile t+1
            if t + 1 in qpre:
                q_burst(t + 1)
                if t + 2 < S // 1024:
                    q_prefetch(t + 2)
            if not DEFER:
                norm_emit(*prev)
                for b in range(4):
                    wo_emit(t * 4 + b)

        if DEFER:
            norm_emit(*prev)
            for b in range(4):
                wo_emit(prev[0] * 4 + b)

    nc.compile()
    return nc


def _prep_inputs(Q, K, V, W_Q, W_K, W_V, W_O):
    Q = np.asarray(Q, dtype=np.float32)
    K = np.asarray(K, dtype=np.float32)
    V = np.asarray(V, dtype=np.float32)
    W_Q = np.asarray(W_Q, dtype=np.float32)
    W_K = np.asarray(W_K, dtype=np.float32)
    W_V = np.asarray(W_V, dtype=np.float32)
    W_O = np.asarray(W_O, dtype=np.float32)

    qk_np = np.float16 if FP16_QK else np.float32
    QT = np.ascontiguousarray(Q.T.astype(qk_np))
    KT = np.ascontiguousarray(K.T.astype(qk_np))
    VT = np.ascontiguousarray(V.T.astype(np.float16))
    scale = np.float32(0.125)  # 1/sqrt(64), exact power of two

    in_maps = []
    for c in range(NCORES):
        hA, hB = 2 * c, 2 * c + 1
        in_maps.append({
            "qt": QT,
            "kt": KT,
            "vt": VT,
            "wq": np.ascontiguousarray(
                np.concatenate([W_Q[hA], W_Q[hB]], axis=1).astype(qk_np)),
            "wk": np.ascontiguousarray(
                np.concatenate([W_K[hA] * scale, W_K[hB] * scale],
                               axis=1).astype(qk_np)),
            "wv": np.ascontiguousarray(
                np.concatenate([W_V[hA], W_V[hB]], axis=1).astype(np.float16)),
            "wo": np.ascontiguousarray(
                W_O[c * 128:(c + 1) * 128, :].astype(np.float16)),
        })
    return in_maps


def kernel(Q, K, V, W_Q, W_K, W_V, W_O):
    global LAST_RESULT
    from concourse.bass_utils import run_bass_kernel_spmd

    S = np.asarray(Q).shape[0]
    nc = _cache.get(S)
    if nc is None:
        nc = _build(S)
        _cache[S] = nc

    in_maps = _prep_inputs(Q, K, V, W_Q, W_K, W_V, W_O)
    res = run_bass_kernel_spmd(nc, in_maps, list(range(NCORES)))
    LAST_RESULT = res
    parts = np.stack([res.results[i]["out"] for i in range(NCORES)])
    return parts.sum(axis=0, dtype=np.float32)
